# revision 1
# baseline (speedup 1.0000x reference)
"""GQA attention kernel for 8 TRN2 NeuronCores.

Sharding: core c = (batch b = c//4, kv-head h = c%4). Each core computes its
batch's projections for its KV head + the 4 query heads of that group, runs
causal attention in S^T layout (softmax reduction folded into the PV matmul
via an appended ones-column on V), and produces a partial output for its
256 columns of Wo. Host sums the 4 partials per batch.

All matmuls run as float32r (1 cycle/row on the PE vs 4 for fp32,
~1.5e-4 relative rounding).
"""
import sys, os
sys.path.insert(0, "/opt/trn_rl_repo")
os.environ.setdefault("MYCRO_LOCAL_CACHE", "1")

import numpy as np
from contextlib import ExitStack

import concourse.bass as bass
import concourse.tile as tile
from concourse import bacc, mybir
from concourse.bass_utils import run_bass_kernel_spmd

F32, F32R = mybir.dt.float32, mybir.dt.float32r
AF = mybir.ActivationFunctionType

B, S, DM = 2, 2048, 1024
H, HKV, DK = 16, 4, 64
G = H // HKV                 # 4 query heads per core
NKT = DM // 128              # 8 dmodel k-tiles
NSQ = S // 512               # 4 sq tiles
NSK = S // 128               # 16 sk tiles
N_CORES = 8

_nc_cache = None


def _build():
    nc = bacc.Bacc("TRN2", target_bir_lowering=False, debug=False)
    inp = {}
    for name, shape in [
        ("xqT", [DM, S]), ("xkT", [DM, S]), ("xvT", [DM, S]),
        ("wqT", [DM, G * DK]), ("wkT", [DM, DK]), ("wvT", [DM, DK]),
        ("woT", [G * DK, DM]),
        ("cos2", [128, S]), ("sin2", [128, S]),
        ("r2T", [128, 128]), ("ident", [64, 64]),
        ("masks", [128, 4 * 512]),
    ]:
        inp[name] = nc.dram_tensor(name, shape, F32, kind="ExternalInput").ap()
    out = nc.dram_tensor("out", [S, DM], F32, kind="ExternalOutput").ap()

    with tile.TileContext(nc) as tc, ExitStack() as ctx:
        const = ctx.enter_context(tc.tile_pool(name="const", bufs=1))
        sb = ctx.enter_context(tc.tile_pool(name="sb", bufs=2))
        sbx = ctx.enter_context(tc.tile_pool(name="sbx", bufs=8))
        ps = ctx.enter_context(tc.tile_pool(name="ps", bufs=3, space="PSUM"))
        ps_acc = ctx.enter_context(tc.tile_pool(name="ps_acc", bufs=2, space="PSUM"))
        ps_tr = ctx.enter_context(tc.tile_pool(name="ps_tr", bufs=2, space="PSUM"))

        def load_const(name, shape, dtype=F32R, eng=None):
            if dtype == F32:
                t = const.tile(shape, F32, tag=name + "_raw")
                nc.sync.dma_start(t[:], inp[name][:])
                return t
            r = const.tile(shape, F32R, tag=name)
            nc.gpsimd.dma_start(r[:], inp[name][:])
            return r

        # weights: DRAM [DM, M] -> SBUF [128, NKT*M] (k-tiles along free dim)
        def load_wT(name, m):
            r = const.tile([128, NKT * m], F32R, tag=name)
            for kt in range(NKT):
                nc.gpsimd.dma_start(r[:, kt * m:(kt + 1) * m],
                                    inp[name][kt * 128:(kt + 1) * 128, :])
            return r

        wq_sb = load_wT("wqT", G * DK)        # [128, 8*256]
        wk_sb = load_wT("wkT", DK)            # [128, 8*64]
        wv_sb = load_wT("wvT", DK)
        wo_sb = const.tile([128, 2 * DM], F32R, tag="wo_sb")
        nc.gpsimd.dma_start(wo_sb[:, 0:DM], inp["woT"][0:128, :])
        nc.gpsimd.dma_start(wo_sb[:, DM:2 * DM], inp["woT"][128:256, :])
        cos_sb = load_const("cos2", [128, S], F32)
        sin_sb = load_const("sin2", [128, S], F32)
        r2_sb = load_const("r2T", [128, 128])
        id_sb = load_const("ident", [64, 64])
        mask_sb = load_const("masks", [128, 4 * 512], F32)

        # persistent activations
        qt = [const.tile([128, S], F32R, tag=f"qt{i}", name=f"qt{i}") for i in range(2)]
        krope = const.tile([64, S], F32R, tag="krope")
        khi = const.tile([128, S], F32R, tag="khi")
        v_sb = const.tile([128, NSK, 65], F32R, tag="v_sb")
        ot = [const.tile([128, S], F32R, tag=f"ot{i}", name=f"ot{i}") for i in range(2)]

        def x_chunk(name, kt, st):
            r = sbx.tile([128, 512], F32R, tag=name + "_r")
            nc.gpsimd.dma_start(r[:],
                                inp[name][kt * 128:(kt + 1) * 128, st * 512:(st + 1) * 512])
            return r

        # ---- Q projection + rope (heads packed 2+2 into qt[0], qt[1])
        for st in range(NSQ):
            xq = [x_chunk("xqT", kt, st) for kt in range(NKT)]
            for half in range(2):
                psQ = ps.tile([128, 512], F32, tag="big")
                for kt in range(NKT):
                    o = kt * G * DK + half * 128
                    nc.tensor.matmul(psQ[:], wq_sb[:, o:o + 128], xq[kt][:],
                                     start=(kt == 0), stop=(kt == NKT - 1))
                qsb = sb.tile([128, 512], F32R, tag="pcopy")
                nc.vector.tensor_copy(qsb[:], psQ[:])
                psRot = ps.tile([128, 512], F32, tag="big")
                nc.tensor.matmul(psRot[:], r2_sb[:], qsb[:], start=True, stop=True)
                t1 = sb.tile([128, 512], F32, tag="t1")
                nc.vector.tensor_mul(t1[:], qsb[:], cos_sb[:, st * 512:(st + 1) * 512])
                t2 = sb.tile([128, 512], F32, tag="t2")
                nc.vector.tensor_mul(t2[:], psRot[:], sin_sb[:, st * 512:(st + 1) * 512])
                nc.vector.tensor_add(qt[half][:, st * 512:(st + 1) * 512], t1[:], t2[:])

        # ---- K + V projections
        for st in range(NSQ):
            xk = [x_chunk("xkT", kt, st) for kt in range(NKT)]
            xv = [x_chunk("xvT", kt, st) for kt in range(NKT)]
            psK = ps.tile([64, 512], F32, tag="big")
            for kt in range(NKT):
                nc.tensor.matmul(psK[:], wk_sb[:, kt * DK:(kt + 1) * DK], xk[kt][:],
                                 start=(kt == 0), stop=(kt == NKT - 1))
            ksb = sb.tile([64, 512], F32R, tag="pcopy")
            nc.vector.tensor_copy(ksb[:], psK[:])
            psRotK = ps.tile([64, 512], F32, tag="big")
            nc.tensor.matmul(psRotK[:], r2_sb[0:64, 0:64], ksb[:], start=True, stop=True)
            k1 = sb.tile([64, 512], F32, tag="t1")
            nc.vector.tensor_mul(k1[:], ksb[:], cos_sb[0:64, st * 512:(st + 1) * 512])
            k2 = sb.tile([64, 512], F32, tag="t2")
            nc.vector.tensor_mul(k2[:], psRotK[:], sin_sb[0:64, st * 512:(st + 1) * 512])
            nc.vector.tensor_add(krope[:, st * 512:(st + 1) * 512], k1[:], k2[:])
            nc.sync.dma_start(khi[64:128, st * 512:(st + 1) * 512],
                              krope[:, st * 512:(st + 1) * 512])

            psVT = ps.tile([64, 512], F32, tag="big")
            for kt in range(NKT):
                nc.tensor.matmul(psVT[:], wv_sb[:, kt * DK:(kt + 1) * DK], xv[kt][:],
                                 start=(kt == 0), stop=(kt == NKT - 1))
            vtsb = sb.tile([64, 512], F32R, tag="pcopy")
            nc.vector.tensor_copy(vtsb[:], psVT[:])
            for j in range(4):
                psVtr = ps_tr.tile([128, 64], F32R, tag="tr")
                nc.tensor.transpose(psVtr[:], vtsb[:, j * 128:(j + 1) * 128], id_sb[:])
                nc.vector.tensor_copy(v_sb[:, st * 4 + j, 0:64], psVtr[:])
        nc.gpsimd.memset(v_sb[:, :, 64:65].bitcast(F32), 1.0)

        # ---- attention: h in 4 query heads, st in 4 sq tiles (causal sk range)
        for h in range(G):
            half, sub = h // 2, h % 2
            for st in range(NSQ):
                psO = ps_acc.tile([65, 512], F32, tag="acc")
                nsk = 4 * st + 4
                for skt in range(nsk):
                    di = skt - 4 * st            # >=0 on diagonal tiles
                    psS = ps.tile([128, 512], F32, tag="big")
                    if sub == 0:
                        lhsT = krope[:, skt * 128:(skt + 1) * 128]
                        rhs = qt[half][0:64, st * 512:(st + 1) * 512]
                    else:
                        lhsT = khi[64:128, skt * 128:(skt + 1) * 128]
                        rhs = qt[half][64:128, st * 512:(st + 1) * 512]
                    nc.tensor.matmul(psS[:], lhsT, rhs, start=True, stop=True)
                    pt2 = sb.tile([128, 512], F32R, tag="pt2")
                    if di >= 0:
                        pt = sb.tile([128, 512], F32, tag="pt")
                        nc.scalar.activation(pt[:], psS[:], AF.Exp)
                        nc.vector.tensor_mul(pt2[:], pt[:],
                                             mask_sb[:, di * 512:(di + 1) * 512])
                    else:
                        nc.scalar.activation(pt2[:], psS[:], AF.Exp)
                    nc.tensor.matmul(psO[:], v_sb[:, skt, :], pt2[:],
                                     start=(skt == 0), stop=(skt == nsk - 1))
                recip = sb.tile([128, 512], F32, tag="recip")
                nc.vector.reciprocal(recip[64:65, :], psO[64:65, :])
                recip0 = sb.tile([1, 512], F32, tag="recip0")
                nc.sync.dma_start(recip0[:], recip[64:65, :])
                bcast = sb.tile([64, 512], F32, tag="bcast")
                nc.gpsimd.partition_broadcast(bcast[:], recip0[:])
                if sub == 0:
                    nc.vector.tensor_mul(ot[half][0:64, st * 512:(st + 1) * 512],
                                         psO[0:64, :], bcast[:])
                else:
                    tmp = sb.tile([64, 512], F32R, tag="otmp")
                    nc.vector.tensor_mul(tmp[:], psO[0:64, :], bcast[:])
                    nc.sync.dma_start(ot[half][64:128, st * 512:(st + 1) * 512], tmp[:])

        # ---- output projection
        for st in range(S // 128):
            for dt in range(2):
                psF = ps.tile([128, 512], F32, tag="big")
                nc.tensor.matmul(psF[:], ot[0][:, st * 128:(st + 1) * 128],
                                 wo_sb[:, dt * 512:(dt + 1) * 512],
                                 start=True, stop=False)
                nc.tensor.matmul(psF[:], ot[1][:, st * 128:(st + 1) * 128],
                                 wo_sb[:, DM + dt * 512:DM + (dt + 1) * 512],
                                 start=False, stop=True)
                osb = sb.tile([128, 512], F32, tag="osb")
                nc.scalar.copy(osb[:], psF[:])
                nc.sync.dma_start(out[st * 128:(st + 1) * 128,
                                      dt * 512:(dt + 1) * 512], osb[:])

    nc.compile()
    return nc


def _host_inputs(query, key, value, Wq, Wk, Wv, Wo):
    inv_freq = 1.0 / (10000.0 ** (np.arange(0, DK, 2, dtype=np.float64) / DK))
    t = np.arange(S, dtype=np.float64)
    freqs = np.einsum("s,f->sf", t, inv_freq)
    emb = np.concatenate([freqs, freqs], axis=-1)
    cos = np.cos(emb).astype(np.float32).T.copy()   # [64, S]
    sin = np.sin(emb).astype(np.float32).T.copy()
    cos2 = np.concatenate([cos, cos], axis=0).copy()
    sin2 = np.concatenate([sin, sin], axis=0).copy()
    R = np.zeros((DK, DK), np.float32)
    half = DK // 2
    for d in range(half):
        R[d, d + half] = -1.0
        R[d + half, d] = 1.0
    r2T = np.zeros((128, 128), np.float32)
    r2T[0:64, 0:64] = R.T
    r2T[64:128, 64:128] = R.T
    ident = np.eye(64, dtype=np.float32)
    masks = np.zeros((128, 4 * 512), np.float32)
    rr = np.arange(128)[:, None]
    cc = np.arange(512)[None, :]
    for i in range(4):
        masks[:, i * 512:(i + 1) * 512] = (rr <= cc - 128 * i).astype(np.float32)

    in_maps = []
    for c in range(N_CORES):
        b, h = c // HKV, c % HKV
        in_maps.append({
            "xqT": np.ascontiguousarray(query[b].T),
            "xkT": np.ascontiguousarray(key[b].T),
            "xvT": np.ascontiguousarray(value[b].T),
            "wqT": np.ascontiguousarray((Wq[h * G * DK:(h + 1) * G * DK, :] * 0.125).T),
            "wkT": np.ascontiguousarray(Wk[h * DK:(h + 1) * DK, :].T),
            "wvT": np.ascontiguousarray(Wv[h * DK:(h + 1) * DK, :].T),
            "woT": np.ascontiguousarray(Wo[:, h * G * DK:(h + 1) * G * DK].T),
            "cos2": cos2, "sin2": sin2, "r2T": r2T, "ident": ident, "masks": masks,
        })
    return in_maps


def kernel(query, key, value, Wq, Wk, Wv, Wo):
    global _nc_cache
    query, key, value = (np.asarray(a, np.float32) for a in (query, key, value))
    Wq, Wk, Wv, Wo = (np.asarray(a, np.float32) for a in (Wq, Wk, Wv, Wo))
    in_maps = _host_inputs(query, key, value, Wq, Wk, Wv, Wo)
    if _nc_cache is None:
        _nc_cache = _build()
    res = run_bass_kernel_spmd(_nc_cache, in_maps, list(range(N_CORES)))
    out = np.zeros((B, S, DM), np.float32)
    for c in range(N_CORES):
        out[c // HKV] += res.results[c]["out"]
    return out



# revision 2
# speedup vs baseline: 7.4607x; 7.4607x over previous
"""GQA attention kernel for 8 TRN2 NeuronCores (axon PJRT path).

The wall-clock of a call is dominated by host<->device transfer over the
axon tunnel (~60 MB/s), so the design minimizes wire bytes:

- Sharding: core c = (batch b = c//4, kv-head h = c%4).
- Each core receives only a disjoint fp16 slice of the activations
  (its batch's seq rows [h*512:(h+1)*512], pre-transposed to [3*1024, 512])
  plus its fp16 weight slices. An in-kernel AllGather over the 4-core batch
  group rebuilds the full transposed activations on device (NeuronLink).
- Per-core attention (4 query heads of one KV group) runs as in the
  baseline: causal S^T layout, softmax reduction folded into the PV matmul
  via an appended ones-column on V, f32r tensor ops. Projections consume
  fp16 operands directly (fp32 PSUM accumulation).
- The per-core partial output (its 256 columns of Wo) is summed across the
  group with an in-kernel ReduceScatter; each core emits a disjoint fp16
  [512, 1024] slice of the final output.
- RoPE tables, masks, identity/rotation matrices and the output zero
  buffers are cached on device once; the jitted shard_map callable is
  cached too, so a warm call ships only ~36 MB fp16 in and ~8 MB fp16 out.
"""
import sys, os
sys.path.insert(0, "/opt/trn_rl_repo")
os.environ.setdefault("MYCRO_LOCAL_CACHE", "1")

import numpy as np
from contextlib import ExitStack

import concourse.bass as bass
import concourse.tile as tile
from concourse import bacc, mybir
import jax
from jax.sharding import Mesh, PartitionSpec, NamedSharding
from jax.experimental.shard_map import shard_map
from concourse.bass2jax import (
    _bass_exec_p,
    install_neuronx_cc_hook,
    partition_id_tensor,
)

F32, F32R, FP16 = mybir.dt.float32, mybir.dt.float32r, mybir.dt.float16
AF = mybir.ActivationFunctionType

B, S, DM = 2, 2048, 1024
H, HKV, DK = 16, 4, 64
G = H // HKV                 # 4 query heads per core
NKT = DM // 128              # 8 dmodel k-tiles
NSQ = S // 512               # 4 sq tiles
NSK = S // 128               # 16 sk tiles
N_CORES = 8
GROUPS = [[0, 1, 2, 3], [4, 5, 6, 7]]
SLOC = S // G                # 512: seq rows shipped per core
XROWS = 3 * DM               # 3072: q|k|v transposed rows per core slice

_runtime = None


def _build():
    nc = bacc.Bacc("TRN2", target_bir_lowering=False, debug=False,
                   num_devices=N_CORES)
    inp = {}
    for name, shape, dt in [
        ("xsT", [XROWS, SLOC], FP16),      # [q|k|v].T slice, local seq cols
        ("wqT", [DM, G * DK], FP16),       # pre-scaled by 0.125
        ("wkT", [DM, DK], FP16),
        ("wvT", [DM, DK], FP16),
        ("woT", [G * DK, DM], FP16),
        ("cos2", [128, S], F32),
        ("sin2", [128, S], F32),
        ("r2T", [128, 128], F32),
        ("ident", [64, 64], F32),
        ("masks", [128, 4 * 512], F32),
    ]:
        inp[name] = nc.dram_tensor(name, shape, dt, kind="ExternalInput").ap()
    out = nc.dram_tensor("out", [SLOC, DM], FP16, kind="ExternalOutput").ap()

    xsT_b = nc.dram_tensor("xsT_b", [XROWS, SLOC], FP16, kind="Internal").ap()
    xgT = nc.dram_tensor("xgT", [G * XROWS, SLOC], FP16, kind="Internal").ap()
    partial = nc.dram_tensor("partial", [S, DM], F32, kind="Internal").ap()
    rsout = nc.dram_tensor("rsout", [SLOC, DM], F32, kind="Internal").ap()

    with tile.TileContext(nc) as tc, ExitStack() as ctx:
        const = ctx.enter_context(tc.tile_pool(name="const", bufs=1))
        sb = ctx.enter_context(tc.tile_pool(name="sb", bufs=2))
        sbx = ctx.enter_context(tc.tile_pool(name="sbx", bufs=8))
        ps = ctx.enter_context(tc.tile_pool(name="ps", bufs=3, space="PSUM"))
        ps_acc = ctx.enter_context(tc.tile_pool(name="ps_acc", bufs=2, space="PSUM"))
        ps_tr = ctx.enter_context(tc.tile_pool(name="ps_tr", bufs=2, space="PSUM"))

        # gather the full transposed activations for this batch across the
        # 4-core group: member h contributed seq cols [h*512:(h+1)*512]
        nc.gpsimd.dma_start(xsT_b[:], inp["xsT"][:])
        nc.gpsimd.collective_compute(
            "AllGather", mybir.AluOpType.bypass, replica_groups=GROUPS,
            ins=[xsT_b[:]], outs=[xgT[:]],
        )

        def load_const(name, shape, dtype=F32R):
            if dtype == F32:
                t = const.tile(shape, F32, tag=name + "_raw")
                nc.sync.dma_start(t[:], inp[name][:])
                return t
            r = const.tile(shape, F32R, tag=name)
            nc.gpsimd.dma_start(r[:], inp[name][:])
            return r

        # weights: DRAM [DM, M] fp16 -> SBUF [128, NKT*M] (k-tiles on free dim)
        def load_wT(name, m):
            r = const.tile([128, NKT * m], FP16, tag=name)
            for kt in range(NKT):
                nc.gpsimd.dma_start(r[:, kt * m:(kt + 1) * m],
                                    inp[name][kt * 128:(kt + 1) * 128, :])
            return r

        wq_sb = load_wT("wqT", G * DK)        # [128, 8*256]
        wk_sb = load_wT("wkT", DK)            # [128, 8*64]
        wv_sb = load_wT("wvT", DK)
        wo_sb = const.tile([128, 2 * DM], FP16, tag="wo_sb")
        nc.gpsimd.dma_start(wo_sb[:, 0:DM], inp["woT"][0:128, :])
        nc.gpsimd.dma_start(wo_sb[:, DM:2 * DM], inp["woT"][128:256, :])
        cos_sb = load_const("cos2", [128, S], F32)
        sin_sb = load_const("sin2", [128, S], F32)
        r2_sb = load_const("r2T", [128, 128])
        id_sb = load_const("ident", [64, 64])
        mask_sb = load_const("masks", [128, 4 * 512], F32)

        # persistent activations
        qt = [const.tile([128, S], F32R, tag=f"qt{i}", name=f"qt{i}") for i in range(2)]
        krope = const.tile([64, S], F32R, tag="krope")
        khi = const.tile([128, S], F32R, tag="khi")
        v_sb = const.tile([128, NSK, 65], F32R, tag="v_sb")
        ot = [const.tile([128, S], FP16, tag=f"ot{i}", name=f"ot{i}") for i in range(2)]

        # x chunk [128, 512] fp16 from the gathered transposed activations:
        # member st's block holds global seq cols [st*512:(st+1)*512]
        def x_chunk(part, kt, st):
            r = sbx.tile([128, 512], FP16, tag=f"x{part}_r")
            base = st * XROWS + part * DM + kt * 128
            nc.gpsimd.dma_start(r[:], xgT[base:base + 128, :])
            return r

        # ---- Q projection + rope (heads packed 2+2 into qt[0], qt[1])
        for st in range(NSQ):
            xq = [x_chunk(0, kt, st) for kt in range(NKT)]
            for half in range(2):
                psQ = ps.tile([128, 512], F32, tag="big")
                for kt in range(NKT):
                    o = kt * G * DK + half * 128
                    nc.tensor.matmul(psQ[:], wq_sb[:, o:o + 128], xq[kt][:],
                                     start=(kt == 0), stop=(kt == NKT - 1))
                qsb = sb.tile([128, 512], F32R, tag="pcopy")
                nc.vector.tensor_copy(qsb[:], psQ[:])
                psRot = ps.tile([128, 512], F32, tag="big")
                nc.tensor.matmul(psRot[:], r2_sb[:], qsb[:], start=True, stop=True)
                t1 = sb.tile([128, 512], F32, tag="t1")
                nc.vector.tensor_mul(t1[:], qsb[:], cos_sb[:, st * 512:(st + 1) * 512])
                t2 = sb.tile([128, 512], F32, tag="t2")
                nc.vector.tensor_mul(t2[:], psRot[:], sin_sb[:, st * 512:(st + 1) * 512])
                nc.vector.tensor_add(qt[half][:, st * 512:(st + 1) * 512], t1[:], t2[:])

        # ---- K + V projections
        for st in range(NSQ):
            xk = [x_chunk(1, kt, st) for kt in range(NKT)]
            xv = [x_chunk(2, kt, st) for kt in range(NKT)]
            psK = ps.tile([64, 512], F32, tag="big")
            for kt in range(NKT):
                nc.tensor.matmul(psK[:], wk_sb[:, kt * DK:(kt + 1) * DK], xk[kt][:],
                                 start=(kt == 0), stop=(kt == NKT - 1))
            ksb = sb.tile([64, 512], F32R, tag="pcopy")
            nc.vector.tensor_copy(ksb[:], psK[:])
            psRotK = ps.tile([64, 512], F32, tag="big")
            nc.tensor.matmul(psRotK[:], r2_sb[0:64, 0:64], ksb[:], start=True, stop=True)
            k1 = sb.tile([64, 512], F32, tag="t1")
            nc.vector.tensor_mul(k1[:], ksb[:], cos_sb[0:64, st * 512:(st + 1) * 512])
            k2 = sb.tile([64, 512], F32, tag="t2")
            nc.vector.tensor_mul(k2[:], psRotK[:], sin_sb[0:64, st * 512:(st + 1) * 512])
            nc.vector.tensor_add(krope[:, st * 512:(st + 1) * 512], k1[:], k2[:])
            nc.sync.dma_start(khi[64:128, st * 512:(st + 1) * 512],
                              krope[:, st * 512:(st + 1) * 512])

            psVT = ps.tile([64, 512], F32, tag="big")
            for kt in range(NKT):
                nc.tensor.matmul(psVT[:], wv_sb[:, kt * DK:(kt + 1) * DK], xv[kt][:],
                                 start=(kt == 0), stop=(kt == NKT - 1))
            vtsb = sb.tile([64, 512], F32R, tag="pcopy")
            nc.vector.tensor_copy(vtsb[:], psVT[:])
            for j in range(4):
                psVtr = ps_tr.tile([128, 64], F32R, tag="tr")
                nc.tensor.transpose(psVtr[:], vtsb[:, j * 128:(j + 1) * 128], id_sb[:])
                nc.vector.tensor_copy(v_sb[:, st * 4 + j, 0:64], psVtr[:])
        nc.gpsimd.memset(v_sb[:, :, 64:65].bitcast(F32), 1.0)

        # ---- attention: h in 4 query heads, st in 4 sq tiles (causal sk range)
        for h in range(G):
            half, sub = h // 2, h % 2
            for st in range(NSQ):
                psO = ps_acc.tile([65, 512], F32, tag="acc")
                nsk = 4 * st + 4
                for skt in range(nsk):
                    di = skt - 4 * st            # >=0 on diagonal tiles
                    psS = ps.tile([128, 512], F32, tag="big")
                    if sub == 0:
                        lhsT = krope[:, skt * 128:(skt + 1) * 128]
                        rhs = qt[half][0:64, st * 512:(st + 1) * 512]
                    else:
                        lhsT = khi[64:128, skt * 128:(skt + 1) * 128]
                        rhs = qt[half][64:128, st * 512:(st + 1) * 512]
                    nc.tensor.matmul(psS[:], lhsT, rhs, start=True, stop=True)
                    pt2 = sb.tile([128, 512], F32R, tag="pt2")
                    if di >= 0:
                        pt = sb.tile([128, 512], F32, tag="pt")
                        nc.scalar.activation(pt[:], psS[:], AF.Exp)
                        nc.vector.tensor_mul(pt2[:], pt[:],
                                             mask_sb[:, di * 512:(di + 1) * 512])
                    else:
                        nc.scalar.activation(pt2[:], psS[:], AF.Exp)
                    nc.tensor.matmul(psO[:], v_sb[:, skt, :], pt2[:],
                                     start=(skt == 0), stop=(skt == nsk - 1))
                recip = sb.tile([128, 512], F32, tag="recip")
                nc.vector.reciprocal(recip[64:65, :], psO[64:65, :])
                recip0 = sb.tile([1, 512], F32, tag="recip0")
                nc.sync.dma_start(recip0[:], recip[64:65, :])
                bcast = sb.tile([64, 512], F32, tag="bcast")
                nc.gpsimd.partition_broadcast(bcast[:], recip0[:])
                if sub == 0:
                    nc.vector.tensor_mul(ot[half][0:64, st * 512:(st + 1) * 512],
                                         psO[0:64, :], bcast[:])
                else:
                    tmp = sb.tile([64, 512], FP16, tag="otmp")
                    nc.vector.tensor_mul(tmp[:], psO[0:64, :], bcast[:])
                    nc.sync.dma_start(ot[half][64:128, st * 512:(st + 1) * 512], tmp[:])

        # ---- output projection into the fp32 partial buffer
        for st in range(S // 128):
            for dt in range(2):
                psF = ps.tile([128, 512], F32, tag="big")
                nc.tensor.matmul(psF[:], ot[0][:, st * 128:(st + 1) * 128],
                                 wo_sb[:, dt * 512:(dt + 1) * 512],
                                 start=True, stop=False)
                nc.tensor.matmul(psF[:], ot[1][:, st * 128:(st + 1) * 128],
                                 wo_sb[:, DM + dt * 512:DM + (dt + 1) * 512],
                                 start=False, stop=True)
                osb = sb.tile([128, 512], F32, tag="osb")
                nc.scalar.copy(osb[:], psF[:])
                nc.sync.dma_start(partial[st * 128:(st + 1) * 128,
                                          dt * 512:(dt + 1) * 512], osb[:])

        # ---- sum partials across the group; member h keeps seq rows
        # [h*512:(h+1)*512]; emit as fp16
        nc.gpsimd.collective_compute(
            "ReduceScatter", mybir.AluOpType.add, replica_groups=GROUPS,
            ins=[partial[:]], outs=[rsout[:]],
        )
        for i in range(SLOC // 128):
            rs_sb = sb.tile([128, DM], F32, tag="rs_sb")
            nc.sync.dma_start(rs_sb[:], rsout[i * 128:(i + 1) * 128, :])
            rs16 = sb.tile([128, DM], FP16, tag="rs16")
            nc.scalar.copy(rs16[:], rs_sb[:])
            nc.sync.dma_start(out[i * 128:(i + 1) * 128, :], rs16[:])

    nc.compile()
    return nc


def _make_consts():
    inv_freq = 1.0 / (10000.0 ** (np.arange(0, DK, 2, dtype=np.float64) / DK))
    t = np.arange(S, dtype=np.float64)
    freqs = np.einsum("s,f->sf", t, inv_freq)
    emb = np.concatenate([freqs, freqs], axis=-1)
    cos = np.cos(emb).astype(np.float32).T.copy()   # [64, S]
    sin = np.sin(emb).astype(np.float32).T.copy()
    cos2 = np.concatenate([cos, cos], axis=0).copy()
    sin2 = np.concatenate([sin, sin], axis=0).copy()
    R = np.zeros((DK, DK), np.float32)
    half = DK // 2
    for d in range(half):
        R[d, d + half] = -1.0
        R[d + half, d] = 1.0
    r2T = np.zeros((128, 128), np.float32)
    r2T[0:64, 0:64] = R.T
    r2T[64:128, 64:128] = R.T
    ident = np.eye(64, dtype=np.float32)
    masks = np.zeros((128, 4 * 512), np.float32)
    rr = np.arange(128)[:, None]
    cc = np.arange(512)[None, :]
    for i in range(4):
        masks[:, i * 512:(i + 1) * 512] = (rr <= cc - 128 * i).astype(np.float32)
    return {"cos2": cos2, "sin2": sin2, "r2T": r2T, "ident": ident, "masks": masks}


def _init_runtime():
    nc = _build()
    install_neuronx_cc_hook()
    partition_name = nc.partition_id_tensor.name if nc.partition_id_tensor else None
    in_names, out_names, out_avals = [], [], []
    for alloc in nc.m.functions[0].allocations:
        if not isinstance(alloc, mybir.MemoryLocationSet):
            continue
        name = alloc.memorylocations[0].name
        if alloc.kind == "ExternalInput":
            if name != partition_name:
                in_names.append(name)
        elif alloc.kind == "ExternalOutput":
            out_names.append(name)
            out_avals.append(jax.core.ShapedArray(
                tuple(alloc.tensor_shape), mybir.dt.np(alloc.dtype)))
    all_in = list(in_names) + list(out_names)
    if partition_name is not None:
        all_in.append(partition_name)

    def _body(*args):
        operands = list(args)
        if partition_name is not None:
            operands.append(partition_id_tensor())
        return tuple(_bass_exec_p.bind(
            *operands, out_avals=tuple(out_avals), in_names=tuple(all_in),
            out_names=tuple(out_names), lowering_input_output_aliases=(),
            sim_require_finite=True, sim_require_nnan=True, nc=nc))

    mesh = Mesh(np.asarray(jax.devices()[:N_CORES]), ("core",))
    nspec = len(in_names) + len(out_names)
    fn = jax.jit(shard_map(_body, mesh=mesh,
                           in_specs=(PartitionSpec("core"),) * nspec,
                           out_specs=(PartitionSpec("core"),) * len(out_names),
                           check_rep=False))
    sh = NamedSharding(mesh, PartitionSpec("core"))

    consts = _make_consts()
    dev_consts = {
        name: jax.device_put(np.tile(arr, (N_CORES,) + (1,) * (arr.ndim - 1)), sh)
        for name, arr in consts.items()
    }
    dev_zeros = [
        jax.device_put(np.zeros((N_CORES * a.shape[0], *a.shape[1:]), a.dtype), sh)
        for a in out_avals
    ]
    return {
        "fn": fn, "sh": sh, "in_names": in_names,
        "dev_consts": dev_consts, "dev_zeros": dev_zeros,
    }


def _host_arrays(query, key, value, Wq, Wk, Wv, Wo):
    q16 = query.astype(np.float16)
    k16 = key.astype(np.float16)
    v16 = value.astype(np.float16)
    xsT_g = np.empty((N_CORES * XROWS, SLOC), np.float16)
    for c in range(N_CORES):
        b, h = c // HKV, c % HKV
        sl = slice(h * SLOC, (h + 1) * SLOC)
        base = c * XROWS
        xsT_g[base:base + DM, :] = q16[b, sl, :].T
        xsT_g[base + DM:base + 2 * DM, :] = k16[b, sl, :].T
        xsT_g[base + 2 * DM:base + 3 * DM, :] = v16[b, sl, :].T
    wq_g = np.empty((N_CORES * DM, G * DK), np.float16)
    wk_g = np.empty((N_CORES * DM, DK), np.float16)
    wv_g = np.empty((N_CORES * DM, DK), np.float16)
    wo_g = np.empty((N_CORES * G * DK, DM), np.float16)
    for c in range(N_CORES):
        h = c % HKV
        wq_g[c * DM:(c + 1) * DM] = Wq[h * G * DK:(h + 1) * G * DK, :].T * np.float32(0.125)
        wk_g[c * DM:(c + 1) * DM] = Wk[h * DK:(h + 1) * DK, :].T
        wv_g[c * DM:(c + 1) * DM] = Wv[h * DK:(h + 1) * DK, :].T
        wo_g[c * G * DK:(c + 1) * G * DK] = Wo[:, h * G * DK:(h + 1) * G * DK].T
    return {"xsT": xsT_g, "wqT": wq_g, "wkT": wk_g, "wvT": wv_g, "woT": wo_g}


def kernel(query, key, value, Wq, Wk, Wv, Wo):
    global _runtime
    query, key, value = (np.asarray(a, np.float32) for a in (query, key, value))
    Wq, Wk, Wv, Wo = (np.asarray(a, np.float32) for a in (Wq, Wk, Wv, Wo))
    if _runtime is None:
        _runtime = _init_runtime()
    rt = _runtime
    host = _host_arrays(query, key, value, Wq, Wk, Wv, Wo)
    dev = jax.device_put([host[n] for n in ("xsT", "wqT", "wkT", "wvT", "woT")],
                         [rt["sh"]] * 5)
    by_name = dict(zip(("xsT", "wqT", "wkT", "wvT", "woT"), dev))
    by_name.update(rt["dev_consts"])
    args = [by_name[n] for n in rt["in_names"]]
    outs = rt["fn"](*args, *rt["dev_zeros"])
    out_g = np.asarray(outs[0])                    # [8*512, 1024] fp16
    return out_g.reshape(B, S, DM).astype(np.float32)


# revision 7
# speedup vs baseline: 8.2938x; 1.1117x over previous
"""GQA attention kernel for 8 TRN2 NeuronCores (axon PJRT path).

The wall-clock of a call is dominated by host<->device transfer over the
axon tunnel (~60 MB/s), so the design minimizes wire bytes:

- Sharding: core c = (batch b = c//4, kv-head h = c%4).
- Each core receives only a disjoint fp16 slice of the activations
  (its batch's seq rows [h*512:(h+1)*512], pre-transposed to [3*1024, 512])
  plus its fp16 weight slices. An in-kernel AllGather over the 4-core batch
  group rebuilds the full transposed activations on device (NeuronLink).
- Per-core attention (4 query heads of one KV group) runs as in the
  baseline: causal S^T layout, softmax reduction folded into the PV matmul
  via an appended ones-column on V, f32r tensor ops. Projections consume
  fp16 operands directly (fp32 PSUM accumulation).
- The per-core partial output (its 256 columns of Wo) is summed across the
  group with an in-kernel ReduceScatter; each core emits a disjoint fp16
  [512, 1024] slice of the final output.
- RoPE tables, masks, identity/rotation matrices and the output zero
  buffers are cached on device once; the jitted shard_map callable is
  cached too, so a warm call ships only ~36 MB fp16 in and ~8 MB fp16 out.
"""
import sys, os
sys.path.insert(0, "/opt/trn_rl_repo")
os.environ.setdefault("MYCRO_LOCAL_CACHE", "1")

import numpy as np
from contextlib import ExitStack

import concourse.bass as bass
import concourse.tile as tile
from concourse import bacc, mybir
import jax
from jax.sharding import Mesh, PartitionSpec, NamedSharding
from jax.experimental.shard_map import shard_map
from concourse.bass2jax import (
    _bass_exec_p,
    install_neuronx_cc_hook,
    partition_id_tensor,
)

F32, F32R, FP16 = mybir.dt.float32, mybir.dt.float32r, mybir.dt.float16
AF = mybir.ActivationFunctionType

B, S, DM = 2, 2048, 1024
H, HKV, DK = 16, 4, 64
G = H // HKV                 # 4 query heads per core
NKT = DM // 128              # 8 dmodel k-tiles
NSQ = S // 512               # 4 sq tiles
NSK = S // 128               # 16 sk tiles
N_CORES = 8
GROUPS = [[0, 1, 2, 3], [4, 5, 6, 7]]
PAIRS = [[0, 4], [1, 5], [2, 6], [3, 7]]   # same kv-head, other batch
SLOC = S // G                # 512: seq rows shipped per core
XROWS = 3 * DM               # 3072: q|k|v transposed rows per core slice

_runtime = None


def _build():
    nc = bacc.Bacc("TRN2", target_bir_lowering=False, debug=False,
                   num_devices=N_CORES)
    inp = {}
    for name, shape, dt in [
        ("xsT", [XROWS, SLOC], FP16),      # [q|k|v].T slice, local seq cols
        # batch-half of the packed weights [wqT | wkT | wvT]; wq pre-scaled
        # by 0.125; the pair AllGather with the same-head core of the other
        # batch rebuilds the full [DM, 384] block
        ("whq", [DM // 2, G * DK + 2 * DK], FP16),
        ("who", [G * DK // 2, DM], FP16),  # batch-half of woT
        ("cos2", [128, S], F32),
        ("sin2", [128, S], F32),
        ("r2T", [128, 128], F32),
        ("ident", [64, 64], F32),
        ("masks", [128, 4 * 512], F32),
    ]:
        inp[name] = nc.dram_tensor(name, shape, dt, kind="ExternalInput").ap()
    out = nc.dram_tensor("out", [SLOC, DM], FP16, kind="ExternalOutput").ap()

    WCOL = G * DK + 2 * DK                 # 384
    xsT_b = nc.dram_tensor("xsT_b", [XROWS, SLOC], FP16, kind="Internal").ap()
    xgT = nc.dram_tensor("xgT", [G * XROWS, SLOC], FP16, kind="Internal").ap()
    whq_b = nc.dram_tensor("whq_b", [DM // 2, WCOL], FP16, kind="Internal").ap()
    whq_g = nc.dram_tensor("whq_g", [DM, WCOL], FP16, kind="Internal").ap()
    who_b = nc.dram_tensor("who_b", [G * DK // 2, DM], FP16, kind="Internal").ap()
    who_g = nc.dram_tensor("who_g", [G * DK, DM], FP16, kind="Internal").ap()
    partial = nc.dram_tensor("partial", [S, DM], F32, kind="Internal").ap()
    rsout = nc.dram_tensor("rsout", [SLOC, DM], F32, kind="Internal").ap()

    with tile.TileContext(nc) as tc, ExitStack() as ctx:
        const = ctx.enter_context(tc.tile_pool(name="const", bufs=1))
        sb = ctx.enter_context(tc.tile_pool(name="sb", bufs=2))
        sbx = ctx.enter_context(tc.tile_pool(name="sbx", bufs=8))
        ps = ctx.enter_context(tc.tile_pool(name="ps", bufs=3, space="PSUM"))
        ps_acc = ctx.enter_context(tc.tile_pool(name="ps_acc", bufs=2, space="PSUM"))
        ps_tr = ctx.enter_context(tc.tile_pool(name="ps_tr", bufs=2, space="PSUM"))

        # rebuild full weights from the batch-halves (pair = same kv-head,
        # other batch), then gather the full transposed activations for this
        # batch across the 4-core group: member h contributed seq cols
        # [h*512:(h+1)*512]
        nc.gpsimd.dma_start(whq_b[:], inp["whq"][:])
        nc.gpsimd.collective_compute(
            "AllGather", mybir.AluOpType.bypass, replica_groups=PAIRS,
            ins=[whq_b[:]], outs=[whq_g[:]],
        )
        nc.gpsimd.dma_start(who_b[:], inp["who"][:])
        nc.gpsimd.collective_compute(
            "AllGather", mybir.AluOpType.bypass, replica_groups=PAIRS,
            ins=[who_b[:]], outs=[who_g[:]],
        )
        nc.gpsimd.dma_start(xsT_b[:], inp["xsT"][:])
        nc.gpsimd.collective_compute(
            "AllGather", mybir.AluOpType.bypass, replica_groups=GROUPS,
            ins=[xsT_b[:]], outs=[xgT[:]],
        )

        def load_const(name, shape, dtype=F32R):
            if dtype == F32:
                t = const.tile(shape, F32, tag=name + "_raw")
                nc.sync.dma_start(t[:], inp[name][:])
                return t
            r = const.tile(shape, F32R, tag=name)
            nc.gpsimd.dma_start(r[:], inp[name][:])
            return r

        # weights: whq_g [DM, 384] fp16 -> SBUF [128, NKT*M] (k-tiles on free
        # dim); columns 0:256 wq, 256:320 wk, 320:384 wv
        def load_wT(col0, m, tag):
            r = const.tile([128, NKT * m], FP16, tag=tag)
            for kt in range(NKT):
                nc.gpsimd.dma_start(r[:, kt * m:(kt + 1) * m],
                                    whq_g[kt * 128:(kt + 1) * 128,
                                          col0:col0 + m])
            return r

        wq_sb = load_wT(0, G * DK, "wq_sb")           # [128, 8*256]
        wk_sb = load_wT(G * DK, DK, "wk_sb")          # [128, 8*64]
        wv_sb = load_wT(G * DK + DK, DK, "wv_sb")
        wo_sb = const.tile([128, 2 * DM], FP16, tag="wo_sb")
        nc.gpsimd.dma_start(wo_sb[:, 0:DM], who_g[0:128, :])
        nc.gpsimd.dma_start(wo_sb[:, DM:2 * DM], who_g[128:256, :])
        cos_sb = load_const("cos2", [128, S], F32)
        sin_sb = load_const("sin2", [128, S], F32)
        r2_sb = load_const("r2T", [128, 128])
        id_sb = load_const("ident", [64, 64])
        mask_sb = load_const("masks", [128, 4 * 512], F32)

        # persistent activations
        qt = [const.tile([128, S], F32R, tag=f"qt{i}", name=f"qt{i}") for i in range(2)]
        krope = const.tile([64, S], F32R, tag="krope")
        khi = const.tile([128, S], F32R, tag="khi")
        v_sb = const.tile([128, NSK, 65], F32R, tag="v_sb")
        ot = [const.tile([128, S], FP16, tag=f"ot{i}", name=f"ot{i}") for i in range(2)]

        # x chunk [128, 512] fp16 from the gathered transposed activations:
        # member st's block holds global seq cols [st*512:(st+1)*512]
        def x_chunk(part, kt, st):
            r = sbx.tile([128, 512], FP16, tag=f"x{part}_r")
            base = st * XROWS + part * DM + kt * 128
            nc.gpsimd.dma_start(r[:], xgT[base:base + 128, :])
            return r

        # ---- Q projection + rope (heads packed 2+2 into qt[0], qt[1])
        for st in range(NSQ):
            xq = [x_chunk(0, kt, st) for kt in range(NKT)]
            for half in range(2):
                psQ = ps.tile([128, 512], F32, tag="big")
                for kt in range(NKT):
                    o = kt * G * DK + half * 128
                    nc.tensor.matmul(psQ[:], wq_sb[:, o:o + 128], xq[kt][:],
                                     start=(kt == 0), stop=(kt == NKT - 1))
                qsb = sb.tile([128, 512], F32R, tag="pcopy")
                nc.vector.tensor_copy(qsb[:], psQ[:])
                psRot = ps.tile([128, 512], F32, tag="big")
                nc.tensor.matmul(psRot[:], r2_sb[:], qsb[:], start=True, stop=True)
                t1 = sb.tile([128, 512], F32, tag="t1")
                nc.vector.tensor_mul(t1[:], qsb[:], cos_sb[:, st * 512:(st + 1) * 512])
                t2 = sb.tile([128, 512], F32, tag="t2")
                nc.vector.tensor_mul(t2[:], psRot[:], sin_sb[:, st * 512:(st + 1) * 512])
                nc.vector.tensor_add(qt[half][:, st * 512:(st + 1) * 512], t1[:], t2[:])

        # ---- K + V projections
        for st in range(NSQ):
            xk = [x_chunk(1, kt, st) for kt in range(NKT)]
            xv = [x_chunk(2, kt, st) for kt in range(NKT)]
            psK = ps.tile([64, 512], F32, tag="big")
            for kt in range(NKT):
                nc.tensor.matmul(psK[:], wk_sb[:, kt * DK:(kt + 1) * DK], xk[kt][:],
                                 start=(kt == 0), stop=(kt == NKT - 1))
            ksb = sb.tile([64, 512], F32R, tag="pcopy")
            nc.vector.tensor_copy(ksb[:], psK[:])
            psRotK = ps.tile([64, 512], F32, tag="big")
            nc.tensor.matmul(psRotK[:], r2_sb[0:64, 0:64], ksb[:], start=True, stop=True)
            k1 = sb.tile([64, 512], F32, tag="t1")
            nc.vector.tensor_mul(k1[:], ksb[:], cos_sb[0:64, st * 512:(st + 1) * 512])
            k2 = sb.tile([64, 512], F32, tag="t2")
            nc.vector.tensor_mul(k2[:], psRotK[:], sin_sb[0:64, st * 512:(st + 1) * 512])
            nc.vector.tensor_add(krope[:, st * 512:(st + 1) * 512], k1[:], k2[:])
            nc.sync.dma_start(khi[64:128, st * 512:(st + 1) * 512],
                              krope[:, st * 512:(st + 1) * 512])

            psVT = ps.tile([64, 512], F32, tag="big")
            for kt in range(NKT):
                nc.tensor.matmul(psVT[:], wv_sb[:, kt * DK:(kt + 1) * DK], xv[kt][:],
                                 start=(kt == 0), stop=(kt == NKT - 1))
            vtsb = sb.tile([64, 512], F32R, tag="pcopy")
            nc.vector.tensor_copy(vtsb[:], psVT[:])
            for j in range(4):
                psVtr = ps_tr.tile([128, 64], F32R, tag="tr")
                nc.tensor.transpose(psVtr[:], vtsb[:, j * 128:(j + 1) * 128], id_sb[:])
                nc.vector.tensor_copy(v_sb[:, st * 4 + j, 0:64], psVtr[:])
        nc.gpsimd.memset(v_sb[:, :, 64:65].bitcast(F32), 1.0)

        # ---- attention: h in 4 query heads, st in 4 sq tiles (causal sk range)
        for h in range(G):
            half, sub = h // 2, h % 2
            for st in range(NSQ):
                psO = ps_acc.tile([65, 512], F32, tag="acc")
                nsk = 4 * st + 4
                for skt in range(nsk):
                    di = skt - 4 * st            # >=0 on diagonal tiles
                    psS = ps.tile([128, 512], F32, tag="big")
                    if sub == 0:
                        lhsT = krope[:, skt * 128:(skt + 1) * 128]
                        rhs = qt[half][0:64, st * 512:(st + 1) * 512]
                    else:
                        lhsT = khi[64:128, skt * 128:(skt + 1) * 128]
                        rhs = qt[half][64:128, st * 512:(st + 1) * 512]
                    nc.tensor.matmul(psS[:], lhsT, rhs, start=True, stop=True)
                    pt2 = sb.tile([128, 512], F32R, tag="pt2")
                    if di >= 0:
                        pt = sb.tile([128, 512], F32, tag="pt")
                        nc.scalar.activation(pt[:], psS[:], AF.Exp)
                        nc.vector.tensor_mul(pt2[:], pt[:],
                                             mask_sb[:, di * 512:(di + 1) * 512])
                    else:
                        nc.scalar.activation(pt2[:], psS[:], AF.Exp)
                    nc.tensor.matmul(psO[:], v_sb[:, skt, :], pt2[:],
                                     start=(skt == 0), stop=(skt == nsk - 1))
                recip = sb.tile([128, 512], F32, tag="recip")
                nc.vector.reciprocal(recip[64:65, :], psO[64:65, :])
                recip0 = sb.tile([1, 512], F32, tag="recip0")
                nc.sync.dma_start(recip0[:], recip[64:65, :])
                bcast = sb.tile([64, 512], F32, tag="bcast")
                nc.gpsimd.partition_broadcast(bcast[:], recip0[:])
                if sub == 0:
                    nc.vector.tensor_mul(ot[half][0:64, st * 512:(st + 1) * 512],
                                         psO[0:64, :], bcast[:])
                else:
                    tmp = sb.tile([64, 512], FP16, tag="otmp")
                    nc.vector.tensor_mul(tmp[:], psO[0:64, :], bcast[:])
                    nc.sync.dma_start(ot[half][64:128, st * 512:(st + 1) * 512], tmp[:])

        # ---- output projection into the fp32 partial buffer
        for st in range(S // 128):
            for dt in range(2):
                psF = ps.tile([128, 512], F32, tag="big")
                nc.tensor.matmul(psF[:], ot[0][:, st * 128:(st + 1) * 128],
                                 wo_sb[:, dt * 512:(dt + 1) * 512],
                                 start=True, stop=False)
                nc.tensor.matmul(psF[:], ot[1][:, st * 128:(st + 1) * 128],
                                 wo_sb[:, DM + dt * 512:DM + (dt + 1) * 512],
                                 start=False, stop=True)
                osb = sb.tile([128, 512], F32, tag="osb")
                nc.scalar.copy(osb[:], psF[:])
                nc.sync.dma_start(partial[st * 128:(st + 1) * 128,
                                          dt * 512:(dt + 1) * 512], osb[:])

        # ---- sum partials across the group; member h keeps seq rows
        # [h*512:(h+1)*512]; emit as fp16
        nc.gpsimd.collective_compute(
            "ReduceScatter", mybir.AluOpType.add, replica_groups=GROUPS,
            ins=[partial[:]], outs=[rsout[:]],
        )
        for i in range(SLOC // 128):
            rs_sb = sb.tile([128, DM], F32, tag="rs_sb")
            nc.sync.dma_start(rs_sb[:], rsout[i * 128:(i + 1) * 128, :])
            rs16 = sb.tile([128, DM], FP16, tag="rs16")
            nc.scalar.copy(rs16[:], rs_sb[:])
            nc.sync.dma_start(out[i * 128:(i + 1) * 128, :], rs16[:])

    nc.compile()
    return nc


def _make_consts():
    inv_freq = 1.0 / (10000.0 ** (np.arange(0, DK, 2, dtype=np.float64) / DK))
    t = np.arange(S, dtype=np.float64)
    freqs = np.einsum("s,f->sf", t, inv_freq)
    emb = np.concatenate([freqs, freqs], axis=-1)
    cos = np.cos(emb).astype(np.float32).T.copy()   # [64, S]
    sin = np.sin(emb).astype(np.float32).T.copy()
    cos2 = np.concatenate([cos, cos], axis=0).copy()
    sin2 = np.concatenate([sin, sin], axis=0).copy()
    R = np.zeros((DK, DK), np.float32)
    half = DK // 2
    for d in range(half):
        R[d, d + half] = -1.0
        R[d + half, d] = 1.0
    r2T = np.zeros((128, 128), np.float32)
    r2T[0:64, 0:64] = R.T
    r2T[64:128, 64:128] = R.T
    ident = np.eye(64, dtype=np.float32)
    masks = np.zeros((128, 4 * 512), np.float32)
    rr = np.arange(128)[:, None]
    cc = np.arange(512)[None, :]
    for i in range(4):
        masks[:, i * 512:(i + 1) * 512] = (rr <= cc - 128 * i).astype(np.float32)
    return {"cos2": cos2, "sin2": sin2, "r2T": r2T, "ident": ident, "masks": masks}


def _init_runtime():
    nc = _build()
    install_neuronx_cc_hook()
    partition_name = nc.partition_id_tensor.name if nc.partition_id_tensor else None
    in_names, out_names, out_avals = [], [], []
    for alloc in nc.m.functions[0].allocations:
        if not isinstance(alloc, mybir.MemoryLocationSet):
            continue
        name = alloc.memorylocations[0].name
        if alloc.kind == "ExternalInput":
            if name != partition_name:
                in_names.append(name)
        elif alloc.kind == "ExternalOutput":
            out_names.append(name)
            out_avals.append(jax.core.ShapedArray(
                tuple(alloc.tensor_shape), mybir.dt.np(alloc.dtype)))
    all_in = list(in_names) + list(out_names)
    if partition_name is not None:
        all_in.append(partition_name)

    def _body(*args):
        operands = list(args)
        if partition_name is not None:
            operands.append(partition_id_tensor())
        return tuple(_bass_exec_p.bind(
            *operands, out_avals=tuple(out_avals), in_names=tuple(all_in),
            out_names=tuple(out_names), lowering_input_output_aliases=(),
            sim_require_finite=True, sim_require_nnan=True, nc=nc))

    mesh = Mesh(np.asarray(jax.devices()[:N_CORES]), ("core",))
    nspec = len(in_names) + len(out_names)
    fn = jax.jit(shard_map(_body, mesh=mesh,
                           in_specs=(PartitionSpec("core"),) * nspec,
                           out_specs=(PartitionSpec("core"),) * len(out_names),
                           check_rep=False))
    sh = NamedSharding(mesh, PartitionSpec("core"))

    consts = _make_consts()
    dev_consts = {
        name: jax.device_put(np.tile(arr, (N_CORES,) + (1,) * (arr.ndim - 1)), sh)
        for name, arr in consts.items()
    }
    dev_zeros = [
        jax.device_put(np.zeros((N_CORES * a.shape[0], *a.shape[1:]), a.dtype), sh)
        for a in out_avals
    ]
    return {
        "fn": fn, "sh": sh, "in_names": in_names,
        "dev_consts": dev_consts, "dev_zeros": dev_zeros,
    }


def _host_xsT(query, key, value):
    q16 = query.astype(np.float16)
    k16 = key.astype(np.float16)
    v16 = value.astype(np.float16)
    xsT_g = np.empty((N_CORES * XROWS, SLOC), np.float16)
    for c in range(N_CORES):
        b, h = c // HKV, c % HKV
        sl = slice(h * SLOC, (h + 1) * SLOC)
        base = c * XROWS
        xsT_g[base:base + DM, :] = q16[b, sl, :].T
        xsT_g[base + DM:base + 2 * DM, :] = k16[b, sl, :].T
        xsT_g[base + 2 * DM:base + 3 * DM, :] = v16[b, sl, :].T
    return xsT_g


def _host_weights(Wq, Wk, Wv, Wo):
    HD = DM // 2
    WCOL = G * DK + 2 * DK
    whq_g = np.empty((N_CORES * HD, WCOL), np.float16)
    who_g = np.empty((N_CORES * G * DK // 2, DM), np.float16)
    for c in range(N_CORES):
        b, h = c // HKV, c % HKV
        r = slice(b * HD, (b + 1) * HD)            # this core's dmodel half
        blk = whq_g[c * HD:(c + 1) * HD]
        blk[:, 0:G * DK] = Wq[h * G * DK:(h + 1) * G * DK, :].T[r] * np.float32(0.125)
        blk[:, G * DK:G * DK + DK] = Wk[h * DK:(h + 1) * DK, :].T[r]
        blk[:, G * DK + DK:WCOL] = Wv[h * DK:(h + 1) * DK, :].T[r]
        ro = slice(b * G * DK // 2, (b + 1) * G * DK // 2)
        who_g[c * G * DK // 2:(c + 1) * G * DK // 2] = \
            Wo[:, h * G * DK:(h + 1) * G * DK].T[ro]
    return whq_g, who_g


def kernel(query, key, value, Wq, Wk, Wv, Wo):
    global _runtime
    query, key, value = (np.asarray(a, np.float32) for a in (query, key, value))
    Wq, Wk, Wv, Wo = (np.asarray(a, np.float32) for a in (Wq, Wk, Wv, Wo))
    if _runtime is None:
        _runtime = _init_runtime()
    rt = _runtime
    # start streaming the big activation array before building the weight
    # arrays: device_put is async, so the host prep below overlaps the wire
    dev_xs = jax.device_put(_host_xsT(query, key, value), rt["sh"])
    whq_g, who_g = _host_weights(Wq, Wk, Wv, Wo)
    dev_wq, dev_wo = jax.device_put([whq_g, who_g], [rt["sh"]] * 2)
    by_name = {"xsT": dev_xs, "whq": dev_wq, "who": dev_wo}
    by_name.update(rt["dev_consts"])
    args = [by_name[n] for n in rt["in_names"]]
    outs = rt["fn"](*args, *rt["dev_zeros"])
    out_g = np.asarray(outs[0])                    # [8*512, 1024] fp16
    return out_g.reshape(B, S, DM).astype(np.float32)


# revision 8
# speedup vs baseline: 9.0300x; 1.0888x over previous
"""GQA attention kernel for 8 TRN2 NeuronCores (axon PJRT path).

The wall-clock of a call is dominated by host<->device transfer over the
axon tunnel (~60 MB/s), so the design minimizes wire bytes:

- Sharding: core c = (batch b = c//4, kv-head h = c%4).
- Each core receives only a disjoint fp16 slice of the activations
  (its batch's seq rows [h*512:(h+1)*512], pre-transposed to [3*1024, 512])
  plus its fp16 weight slices. An in-kernel AllGather over the 4-core batch
  group rebuilds the full transposed activations on device (NeuronLink).
- Per-core attention (4 query heads of one KV group) runs as in the
  baseline: causal S^T layout, softmax reduction folded into the PV matmul
  via an appended ones-column on V, f32r tensor ops. Projections consume
  fp16 operands directly (fp32 PSUM accumulation).
- The per-core partial output (its 256 columns of Wo) is summed across the
  group with an in-kernel ReduceScatter; each core emits a disjoint fp16
  [512, 1024] slice of the final output.
- RoPE tables, masks, identity/rotation matrices and the output zero
  buffers are cached on device once; the jitted shard_map callable is
  cached too, so a warm call ships only ~36 MB fp16 in and ~8 MB fp16 out.
"""
import sys, os
sys.path.insert(0, "/opt/trn_rl_repo")
os.environ.setdefault("MYCRO_LOCAL_CACHE", "1")

import numpy as np
from contextlib import ExitStack

import concourse.bass as bass
import concourse.tile as tile
from concourse import bacc, mybir
import jax
from jax.sharding import Mesh, PartitionSpec, NamedSharding
from jax.experimental.shard_map import shard_map
from concourse.bass2jax import (
    _bass_exec_p,
    install_neuronx_cc_hook,
    partition_id_tensor,
)

F32, F32R, FP16 = mybir.dt.float32, mybir.dt.float32r, mybir.dt.float16
AF = mybir.ActivationFunctionType

B, S, DM = 2, 2048, 1024
H, HKV, DK = 16, 4, 64
G = H // HKV                 # 4 query heads per core
NKT = DM // 128              # 8 dmodel k-tiles
NSQ = S // 512               # 4 sq tiles
NSK = S // 128               # 16 sk tiles
N_CORES = 8
GROUPS = [[0, 1, 2, 3], [4, 5, 6, 7]]
PAIRS = [[0, 4], [1, 5], [2, 6], [3, 7]]   # same kv-head, other batch
SLOC = S // G                # 512: seq rows shipped per core
XROWS = 3 * DM               # 3072: q|k|v transposed rows per core slice

_runtime = None


def _build():
    nc = bacc.Bacc("TRN2", target_bir_lowering=False, debug=False,
                   num_devices=N_CORES)
    inp = {}
    for name, shape, dt in [
        ("xsT", [XROWS, SLOC], FP16),      # [q|k|v].T slice, local seq cols
        # batch-half of the packed weights [wqT | wkT | wvT]; wq pre-scaled
        # by 0.125; the pair AllGather with the same-head core of the other
        # batch rebuilds the full [DM, 384] block
        ("whq", [DM // 2, G * DK + 2 * DK], FP16),
        ("who", [G * DK // 2, DM], FP16),  # batch-half of woT
        ("cos2", [128, S], F32),
        ("sin2", [128, S], F32),
        ("r2T", [128, 128], F32),
        ("ident", [64, 64], F32),
        ("masks", [128, 4 * 512], F32),
    ]:
        inp[name] = nc.dram_tensor(name, shape, dt, kind="ExternalInput").ap()
    out = nc.dram_tensor("out", [SLOC, DM], FP16, kind="ExternalOutput").ap()

    WCOL = G * DK + 2 * DK                 # 384
    xsT_b = nc.dram_tensor("xsT_b", [XROWS, SLOC], FP16, kind="Internal").ap()
    xgT = nc.dram_tensor("xgT", [G * XROWS, SLOC], FP16, kind="Internal").ap()
    whq_b = nc.dram_tensor("whq_b", [DM // 2, WCOL], FP16, kind="Internal").ap()
    whq_g = nc.dram_tensor("whq_g", [DM, WCOL], FP16, kind="Internal").ap()
    who_b = nc.dram_tensor("who_b", [G * DK // 2, DM], FP16, kind="Internal").ap()
    who_g = nc.dram_tensor("who_g", [G * DK, DM], FP16, kind="Internal").ap()
    partial = nc.dram_tensor("partial", [S, DM], F32, kind="Internal").ap()
    rsout = nc.dram_tensor("rsout", [SLOC, DM], F32, kind="Internal").ap()

    with tile.TileContext(nc) as tc, ExitStack() as ctx:
        const = ctx.enter_context(tc.tile_pool(name="const", bufs=1))
        sb = ctx.enter_context(tc.tile_pool(name="sb", bufs=2))
        sbx = ctx.enter_context(tc.tile_pool(name="sbx", bufs=8))
        ps = ctx.enter_context(tc.tile_pool(name="ps", bufs=3, space="PSUM"))
        ps_acc = ctx.enter_context(tc.tile_pool(name="ps_acc", bufs=2, space="PSUM"))
        ps_tr = ctx.enter_context(tc.tile_pool(name="ps_tr", bufs=2, space="PSUM"))

        # rebuild full weights from the batch-halves (pair = same kv-head,
        # other batch), then gather the full transposed activations for this
        # batch across the 4-core group: member h contributed seq cols
        # [h*512:(h+1)*512]
        nc.gpsimd.dma_start(whq_b[:], inp["whq"][:])
        nc.gpsimd.collective_compute(
            "AllGather", mybir.AluOpType.bypass, replica_groups=PAIRS,
            ins=[whq_b[:]], outs=[whq_g[:]],
        )
        nc.gpsimd.dma_start(who_b[:], inp["who"][:])
        nc.gpsimd.collective_compute(
            "AllGather", mybir.AluOpType.bypass, replica_groups=PAIRS,
            ins=[who_b[:]], outs=[who_g[:]],
        )
        nc.gpsimd.dma_start(xsT_b[:], inp["xsT"][:])
        nc.gpsimd.collective_compute(
            "AllGather", mybir.AluOpType.bypass, replica_groups=GROUPS,
            ins=[xsT_b[:]], outs=[xgT[:]],
        )

        def load_const(name, shape, dtype=F32R):
            if dtype == F32:
                t = const.tile(shape, F32, tag=name + "_raw")
                nc.sync.dma_start(t[:], inp[name][:])
                return t
            r = const.tile(shape, F32R, tag=name)
            nc.gpsimd.dma_start(r[:], inp[name][:])
            return r

        # weights: whq_g [DM, 384] fp16 -> SBUF [128, NKT*M] (k-tiles on free
        # dim); columns 0:256 wq, 256:320 wk, 320:384 wv
        def load_wT(col0, m, tag):
            r = const.tile([128, NKT * m], FP16, tag=tag)
            for kt in range(NKT):
                nc.gpsimd.dma_start(r[:, kt * m:(kt + 1) * m],
                                    whq_g[kt * 128:(kt + 1) * 128,
                                          col0:col0 + m])
            return r

        wq_sb = load_wT(0, G * DK, "wq_sb")           # [128, 8*256]
        wk_sb = load_wT(G * DK, DK, "wk_sb")          # [128, 8*64]
        wv_sb = load_wT(G * DK + DK, DK, "wv_sb")
        wo_sb = const.tile([128, 2 * DM], FP16, tag="wo_sb")
        nc.gpsimd.dma_start(wo_sb[:, 0:DM], who_g[0:128, :])
        nc.gpsimd.dma_start(wo_sb[:, DM:2 * DM], who_g[128:256, :])
        cos_sb = load_const("cos2", [128, S], F32)
        sin_sb = load_const("sin2", [128, S], F32)
        r2_sb = load_const("r2T", [128, 128])
        id_sb = load_const("ident", [64, 64])
        mask_sb = load_const("masks", [128, 4 * 512], F32)

        # persistent activations
        qt = [const.tile([128, S], F32R, tag=f"qt{i}", name=f"qt{i}") for i in range(2)]
        krope = const.tile([64, S], F32R, tag="krope")
        khi = const.tile([128, S], F32R, tag="khi")
        v_sb = const.tile([128, NSK, 65], F32R, tag="v_sb")
        ot = [const.tile([128, S], FP16, tag=f"ot{i}", name=f"ot{i}") for i in range(2)]

        # x chunk [128, 512] fp16 from the gathered transposed activations:
        # member st's block holds global seq cols [st*512:(st+1)*512]
        def x_chunk(part, kt, st):
            r = sbx.tile([128, 512], FP16, tag=f"x{part}_r")
            base = st * XROWS + part * DM + kt * 128
            nc.gpsimd.dma_start(r[:], xgT[base:base + 128, :])
            return r

        # ---- Q projection + rope (heads packed 2+2 into qt[0], qt[1])
        for st in range(NSQ):
            xq = [x_chunk(0, kt, st) for kt in range(NKT)]
            for half in range(2):
                psQ = ps.tile([128, 512], F32, tag="big")
                for kt in range(NKT):
                    o = kt * G * DK + half * 128
                    nc.tensor.matmul(psQ[:], wq_sb[:, o:o + 128], xq[kt][:],
                                     start=(kt == 0), stop=(kt == NKT - 1))
                qsb = sb.tile([128, 512], F32R, tag="pcopy")
                nc.vector.tensor_copy(qsb[:], psQ[:])
                psRot = ps.tile([128, 512], F32, tag="big")
                nc.tensor.matmul(psRot[:], r2_sb[:], qsb[:], start=True, stop=True)
                t1 = sb.tile([128, 512], F32, tag="t1")
                nc.vector.tensor_mul(t1[:], qsb[:], cos_sb[:, st * 512:(st + 1) * 512])
                t2 = sb.tile([128, 512], F32, tag="t2")
                nc.vector.tensor_mul(t2[:], psRot[:], sin_sb[:, st * 512:(st + 1) * 512])
                nc.vector.tensor_add(qt[half][:, st * 512:(st + 1) * 512], t1[:], t2[:])

        # ---- K + V projections
        for st in range(NSQ):
            xk = [x_chunk(1, kt, st) for kt in range(NKT)]
            xv = [x_chunk(2, kt, st) for kt in range(NKT)]
            psK = ps.tile([64, 512], F32, tag="big")
            for kt in range(NKT):
                nc.tensor.matmul(psK[:], wk_sb[:, kt * DK:(kt + 1) * DK], xk[kt][:],
                                 start=(kt == 0), stop=(kt == NKT - 1))
            ksb = sb.tile([64, 512], F32R, tag="pcopy")
            nc.vector.tensor_copy(ksb[:], psK[:])
            psRotK = ps.tile([64, 512], F32, tag="big")
            nc.tensor.matmul(psRotK[:], r2_sb[0:64, 0:64], ksb[:], start=True, stop=True)
            k1 = sb.tile([64, 512], F32, tag="t1")
            nc.vector.tensor_mul(k1[:], ksb[:], cos_sb[0:64, st * 512:(st + 1) * 512])
            k2 = sb.tile([64, 512], F32, tag="t2")
            nc.vector.tensor_mul(k2[:], psRotK[:], sin_sb[0:64, st * 512:(st + 1) * 512])
            nc.vector.tensor_add(krope[:, st * 512:(st + 1) * 512], k1[:], k2[:])
            nc.sync.dma_start(khi[64:128, st * 512:(st + 1) * 512],
                              krope[:, st * 512:(st + 1) * 512])

            psVT = ps.tile([64, 512], F32, tag="big")
            for kt in range(NKT):
                nc.tensor.matmul(psVT[:], wv_sb[:, kt * DK:(kt + 1) * DK], xv[kt][:],
                                 start=(kt == 0), stop=(kt == NKT - 1))
            vtsb = sb.tile([64, 512], F32R, tag="pcopy")
            nc.vector.tensor_copy(vtsb[:], psVT[:])
            for j in range(4):
                psVtr = ps_tr.tile([128, 64], F32R, tag="tr")
                nc.tensor.transpose(psVtr[:], vtsb[:, j * 128:(j + 1) * 128], id_sb[:])
                nc.vector.tensor_copy(v_sb[:, st * 4 + j, 0:64], psVtr[:])
        nc.gpsimd.memset(v_sb[:, :, 64:65].bitcast(F32), 1.0)

        # ---- attention: h in 4 query heads, st in 4 sq tiles (causal sk range)
        for h in range(G):
            half, sub = h // 2, h % 2
            for st in range(NSQ):
                psO = ps_acc.tile([65, 512], F32, tag="acc")
                nsk = 4 * st + 4
                for skt in range(nsk):
                    di = skt - 4 * st            # >=0 on diagonal tiles
                    psS = ps.tile([128, 512], F32, tag="big")
                    if sub == 0:
                        lhsT = krope[:, skt * 128:(skt + 1) * 128]
                        rhs = qt[half][0:64, st * 512:(st + 1) * 512]
                    else:
                        lhsT = khi[64:128, skt * 128:(skt + 1) * 128]
                        rhs = qt[half][64:128, st * 512:(st + 1) * 512]
                    nc.tensor.matmul(psS[:], lhsT, rhs, start=True, stop=True)
                    pt2 = sb.tile([128, 512], F32R, tag="pt2")
                    if di >= 0:
                        pt = sb.tile([128, 512], F32, tag="pt")
                        nc.scalar.activation(pt[:], psS[:], AF.Exp)
                        nc.vector.tensor_mul(pt2[:], pt[:],
                                             mask_sb[:, di * 512:(di + 1) * 512])
                    else:
                        nc.scalar.activation(pt2[:], psS[:], AF.Exp)
                    nc.tensor.matmul(psO[:], v_sb[:, skt, :], pt2[:],
                                     start=(skt == 0), stop=(skt == nsk - 1))
                recip = sb.tile([128, 512], F32, tag="recip")
                nc.vector.reciprocal(recip[64:65, :], psO[64:65, :])
                recip0 = sb.tile([1, 512], F32, tag="recip0")
                nc.sync.dma_start(recip0[:], recip[64:65, :])
                bcast = sb.tile([64, 512], F32, tag="bcast")
                nc.gpsimd.partition_broadcast(bcast[:], recip0[:])
                if sub == 0:
                    nc.vector.tensor_mul(ot[half][0:64, st * 512:(st + 1) * 512],
                                         psO[0:64, :], bcast[:])
                else:
                    tmp = sb.tile([64, 512], FP16, tag="otmp")
                    nc.vector.tensor_mul(tmp[:], psO[0:64, :], bcast[:])
                    nc.sync.dma_start(ot[half][64:128, st * 512:(st + 1) * 512], tmp[:])

        # ---- output projection into the fp32 partial buffer
        for st in range(S // 128):
            for dt in range(2):
                psF = ps.tile([128, 512], F32, tag="big")
                nc.tensor.matmul(psF[:], ot[0][:, st * 128:(st + 1) * 128],
                                 wo_sb[:, dt * 512:(dt + 1) * 512],
                                 start=True, stop=False)
                nc.tensor.matmul(psF[:], ot[1][:, st * 128:(st + 1) * 128],
                                 wo_sb[:, DM + dt * 512:DM + (dt + 1) * 512],
                                 start=False, stop=True)
                osb = sb.tile([128, 512], F32, tag="osb")
                nc.scalar.copy(osb[:], psF[:])
                nc.sync.dma_start(partial[st * 128:(st + 1) * 128,
                                          dt * 512:(dt + 1) * 512], osb[:])

        # ---- sum partials across the group; member h keeps seq rows
        # [h*512:(h+1)*512]; emit as fp16
        nc.gpsimd.collective_compute(
            "ReduceScatter", mybir.AluOpType.add, replica_groups=GROUPS,
            ins=[partial[:]], outs=[rsout[:]],
        )
        for i in range(SLOC // 128):
            rs_sb = sb.tile([128, DM], F32, tag="rs_sb")
            nc.sync.dma_start(rs_sb[:], rsout[i * 128:(i + 1) * 128, :])
            rs16 = sb.tile([128, DM], FP16, tag="rs16")
            nc.scalar.copy(rs16[:], rs_sb[:])
            nc.sync.dma_start(out[i * 128:(i + 1) * 128, :], rs16[:])

    nc.compile()
    return nc


def _make_consts():
    inv_freq = 1.0 / (10000.0 ** (np.arange(0, DK, 2, dtype=np.float64) / DK))
    t = np.arange(S, dtype=np.float64)
    freqs = np.einsum("s,f->sf", t, inv_freq)
    emb = np.concatenate([freqs, freqs], axis=-1)
    cos = np.cos(emb).astype(np.float32).T.copy()   # [64, S]
    sin = np.sin(emb).astype(np.float32).T.copy()
    cos2 = np.concatenate([cos, cos], axis=0).copy()
    sin2 = np.concatenate([sin, sin], axis=0).copy()
    R = np.zeros((DK, DK), np.float32)
    half = DK // 2
    for d in range(half):
        R[d, d + half] = -1.0
        R[d + half, d] = 1.0
    r2T = np.zeros((128, 128), np.float32)
    r2T[0:64, 0:64] = R.T
    r2T[64:128, 64:128] = R.T
    ident = np.eye(64, dtype=np.float32)
    masks = np.zeros((128, 4 * 512), np.float32)
    rr = np.arange(128)[:, None]
    cc = np.arange(512)[None, :]
    for i in range(4):
        masks[:, i * 512:(i + 1) * 512] = (rr <= cc - 128 * i).astype(np.float32)
    return {"cos2": cos2, "sin2": sin2, "r2T": r2T, "ident": ident, "masks": masks}


def _init_runtime():
    nc = _build()
    install_neuronx_cc_hook()
    partition_name = nc.partition_id_tensor.name if nc.partition_id_tensor else None
    in_names, out_names, out_avals = [], [], []
    for alloc in nc.m.functions[0].allocations:
        if not isinstance(alloc, mybir.MemoryLocationSet):
            continue
        name = alloc.memorylocations[0].name
        if alloc.kind == "ExternalInput":
            if name != partition_name:
                in_names.append(name)
        elif alloc.kind == "ExternalOutput":
            out_names.append(name)
            out_avals.append(jax.core.ShapedArray(
                tuple(alloc.tensor_shape), mybir.dt.np(alloc.dtype)))
    all_in = list(in_names) + list(out_names)
    if partition_name is not None:
        all_in.append(partition_name)

    def _body(*args):
        operands = list(args)
        if partition_name is not None:
            operands.append(partition_id_tensor())
        return tuple(_bass_exec_p.bind(
            *operands, out_avals=tuple(out_avals), in_names=tuple(all_in),
            out_names=tuple(out_names), lowering_input_output_aliases=(),
            sim_require_finite=True, sim_require_nnan=True, nc=nc))

    mesh = Mesh(np.asarray(jax.devices()[:N_CORES]), ("core",))
    nspec = len(in_names) + len(out_names)
    fn = jax.jit(shard_map(_body, mesh=mesh,
                           in_specs=(PartitionSpec("core"),) * nspec,
                           out_specs=(PartitionSpec("core"),) * len(out_names),
                           check_rep=False))
    sh = NamedSharding(mesh, PartitionSpec("core"))

    consts = _make_consts()
    dev_consts = {
        name: jax.device_put(np.tile(arr, (N_CORES,) + (1,) * (arr.ndim - 1)), sh)
        for name, arr in consts.items()
    }
    dev_zeros = [
        jax.device_put(np.zeros((N_CORES * a.shape[0], *a.shape[1:]), a.dtype), sh)
        for a in out_avals
    ]
    return {
        "fn": fn, "sh": sh, "in_names": in_names,
        "dev_consts": dev_consts, "dev_zeros": dev_zeros,
    }


def _host_xsT(query, key, value):
    q16 = query.astype(np.float16)
    k16 = key.astype(np.float16)
    v16 = value.astype(np.float16)
    xsT_g = np.empty((N_CORES * XROWS, SLOC), np.float16)
    for c in range(N_CORES):
        b, h = c // HKV, c % HKV
        sl = slice(h * SLOC, (h + 1) * SLOC)
        base = c * XROWS
        xsT_g[base:base + DM, :] = q16[b, sl, :].T
        xsT_g[base + DM:base + 2 * DM, :] = k16[b, sl, :].T
        xsT_g[base + 2 * DM:base + 3 * DM, :] = v16[b, sl, :].T
    return xsT_g


def _host_weights(Wq, Wk, Wv, Wo):
    HD = DM // 2
    WCOL = G * DK + 2 * DK
    whq_g = np.empty((N_CORES * HD, WCOL), np.float16)
    who_g = np.empty((N_CORES * G * DK // 2, DM), np.float16)
    for c in range(N_CORES):
        b, h = c // HKV, c % HKV
        r = slice(b * HD, (b + 1) * HD)            # this core's dmodel half
        blk = whq_g[c * HD:(c + 1) * HD]
        blk[:, 0:G * DK] = Wq[h * G * DK:(h + 1) * G * DK, :].T[r] * np.float32(0.125)
        blk[:, G * DK:G * DK + DK] = Wk[h * DK:(h + 1) * DK, :].T[r]
        blk[:, G * DK + DK:WCOL] = Wv[h * DK:(h + 1) * DK, :].T[r]
        ro = slice(b * G * DK // 2, (b + 1) * G * DK // 2)
        who_g[c * G * DK // 2:(c + 1) * G * DK // 2] = \
            Wo[:, h * G * DK:(h + 1) * G * DK].T[ro]
    return whq_g, who_g


def kernel(query, key, value, Wq, Wk, Wv, Wo):
    global _runtime
    query, key, value = (np.asarray(a, np.float32) for a in (query, key, value))
    Wq, Wk, Wv, Wo = (np.asarray(a, np.float32) for a in (Wq, Wk, Wv, Wo))
    if _runtime is None:
        _runtime = _init_runtime()
    rt = _runtime
    # device_put is async: stream the quick-to-build weight arrays first so
    # the big activation array's host build overlaps their wire time
    whq_g, who_g = _host_weights(Wq, Wk, Wv, Wo)
    dev_wq, dev_wo = jax.device_put([whq_g, who_g], [rt["sh"]] * 2)
    dev_xs = jax.device_put(_host_xsT(query, key, value), rt["sh"])
    by_name = {"xsT": dev_xs, "whq": dev_wq, "who": dev_wo}
    by_name.update(rt["dev_consts"])
    args = [by_name[n] for n in rt["in_names"]]
    outs = rt["fn"](*args, *rt["dev_zeros"])
    out_g = np.asarray(outs[0])                    # [8*512, 1024] fp16
    return out_g.reshape(B, S, DM).astype(np.float32)
